# revision 6
# baseline (speedup 1.0000x reference)
"""Trainium2 Bass kernel for the L1 tensor-product problem.

Math (per batch row b):
  out0e = [x0e*s, CG*(x1o.v)] @ W0e * NORM0E
  out0o = [x0o*s, CG*(x1e.v)] @ W0o * NORM0O
  out1e_c = [CG*x0o*v_c, CG*x1e_c*s, CGC*cross(x1o,v)_c] @ W1e * NORM1E
  out1o_c = [CG*x0e*v_c, CG*x1o_c*s, CGC*cross(x1e,v)_c] @ W1o * NORM1O

Kernel strategy (pure data parallel over batch, 8 cores), v3:
  * bf16 wire + matmul dtype; PSUM accumulates fp32 (rel-err ~4e-3,
    budget 2e-2).
  * DVE is the bottleneck engine (~100% busy): all per-row products run
    as 2x-mode bf16 tensor_tensor ops at the hw max of ~1.92 elem/ns,
    so the schedule minimizes DVE *elements*:
      - pvs: ONE 24T op computes x1 x {s, v0, v1, v2} (the [m,k,t]
        layout puts x1e*s / x1o*s contiguous so the h-path matmul runs
        as a single N=3T instruction, and diag/off-diag blocks land at
        AP-addressable offsets for the k+/k- matmuls and the dots).
      - dots: one DVE add level (dta = d0+d1); the final +d2 is folded
        into the 0e/0o PSUM accumulation as a 4th matmul K-chunk (PE
        has ~13% headroom, DVE has none).
      - t3 = v_c*g on DVE (6T), accumulated via an identity matmul.
  * Unscaled g = x0?' @ Wg runs at the head of each PE seg so sgp is
    ready a full seg before t3 consumes it.
  * The per-parity 1e/1o PSUM is one [128, 3W] region: k+ x3, k- x3,
    one N=3W identity and one N=3W h matmul accumulate into it, and ONE
    Act copy drains it (Act sem ops at ~610ns each were ~27% of the
    Act queue in v2, so fewer, larger copies).
  * First/last tile are split into 4 W=128 subtiles: the DVE starts
    after a quarter-load and the tail drain chain is a quarter-length
    (v2 paid ~5us ramp + ~17us serial drain).
  * Multiplier rows (s,v) reach all 128 partitions via a stride-0
    broadcast DMA read (GpSimd is unusable: its SBUF port contends
    with 2-port DVE ops and its tensor ops trip the chip's utilization
    throttle; PE ones-broadcasts would eat the PE headroom).
"""

import sys

sys.path.insert(0, "/opt/trn_rl_repo")

import numpy as np

import concourse.bass as bass
import concourse.bacc as bacc
import concourse.mybir as mybir
from concourse.bass_utils import run_bass_kernel_spmd
from concourse.tile import TileContext

N_CORES = 8
T = 512  # batch columns per full tile
WEDGE = 128  # subtile width for first/last tile

# irreps: 256x0e + 256x0o + 128x1e + 128x1o
CG = 1.0 / 3.0**0.5
CGC = 1.0 / 6.0**0.5
NORM0E = (1.0 / 384.0) ** 0.5
NORM0O = (1.0 / 384.0) ** 0.5
NORM1E = (3.0 / 512.0) ** 0.5
NORM1O = (3.0 / 512.0) ** 0.5

_BF16 = None


def _bf16():
    global _BF16
    if _BF16 is None:
        import ml_dtypes

        _BF16 = np.dtype(ml_dtypes.bfloat16)
    return _BF16


def _pack_weights(W0e, W0o, W1e, W1o):
    """Fold constants/signs; 22 lhsT chunks [128,128] side by side.

    Order: 0e (kc0m0,kc0m1,kc1m0,kc1m1,kc2m0,kc2m1), 0o (same 6),
    1e (g0,g1,h,k+,k-), 1o (g0,g1,h,k+,k-), identity.
    """
    W0e = W0e.astype(np.float64) * NORM0E
    W0e[256:] *= CG
    W0o = W0o.astype(np.float64) * NORM0O
    W0o[256:] *= CG
    W1e = W1e.astype(np.float64) * NORM1E
    W1e[:384] *= CG
    W1e[384:] *= CGC
    W1o = W1o.astype(np.float64) * NORM1O
    W1o[:384] *= CG
    W1o[384:] *= CGC
    chunks = []
    for W in (W0e, W0o):  # [384, 256]
        for kc in range(3):
            for mc in range(2):
                chunks.append(W[kc * 128 : (kc + 1) * 128, mc * 128 : (mc + 1) * 128])
    for W in (W1e, W1o):  # [512, 128]
        chunks.append(W[0:128, :])      # g0
        chunks.append(W[128:256, :])    # g1
        chunks.append(W[256:384, :])    # h
        chunks.append(W[384:512, :])    # k+
        chunks.append(-W[384:512, :])   # k-
    chunks.append(np.eye(128, dtype=np.float64))  # 22: identity (combine accum)
    packed = np.concatenate(chunks, axis=1)
    return np.ascontiguousarray(packed.astype(_bf16()))


def _prep_shard(in1_s, in2_s):
    """in1 [Bs,1280] -> x [nt, 128, 10*T] bf16; in2 [Bs,4] -> s4 [nt,4,T].

    Chunk order: 0,1=x0e  2,3=x0o  4+c=x1e_c  7+c=x1o_c.
    """
    Bs = in1_s.shape[0]
    nt = Bs // T
    dt = _bf16()
    x = np.empty((nt, 128, 10, T), dt)
    x[:, :, 0:4] = in1_s[:, 0:512].reshape(nt, T, 4, 128).transpose(0, 3, 2, 1)
    x[:, :, 4:7] = in1_s[:, 512:896].reshape(nt, T, 128, 3).transpose(0, 2, 3, 1)
    x[:, :, 7:10] = in1_s[:, 896:1280].reshape(nt, T, 128, 3).transpose(0, 2, 3, 1)
    s4 = np.ascontiguousarray(in2_s.reshape(nt, T, 4).transpose(0, 2, 1).astype(dt))
    return np.ascontiguousarray(x.reshape(nt, 128, 10 * T)), s4


def _post_shard(y):
    """Device y [nt, 128, 10*T] bf16 -> [Bs, 1280] fp32 original layout."""
    nt = y.shape[0]
    y = np.asarray(y).reshape(nt, 128, 10, T).astype(np.float32)
    out = np.empty((nt, T, 1280), np.float32)
    out[:, :, 0:512] = y[:, :, 0:4].transpose(0, 3, 2, 1).reshape(nt, T, 512)
    out[:, :, 512:896] = y[:, :, 4:7].transpose(0, 3, 1, 2).reshape(nt, T, 384)
    out[:, :, 896:1280] = y[:, :, 7:10].transpose(0, 3, 1, 2).reshape(nt, T, 384)
    return out.reshape(nt * T, 1280)


def _segments(nt):
    """Subtile the first and last tile so the pipeline ramp and drain
    chains are WEDGE-length instead of T-length."""
    segs = []
    nw = T // WEDGE
    for j in range(nw):
        segs.append((0, j * WEDGE, WEDGE))
    for t in range(1, nt - 1):
        segs.append((t, 0, T))
    if nt > 1:
        for j in range(nw):
            segs.append((nt - 1, j * WEDGE, WEDGE))
    return segs


def _build_program(Bs):
    assert Bs % T == 0, (Bs, T)
    nt = Bs // T
    bf = mybir.dt.bfloat16
    f32 = mybir.dt.float32

    nc = bacc.Bacc()
    x = nc.declare_dram_parameter("x", [nt, 128, 10 * T], bf, isOutput=False)
    s4 = nc.declare_dram_parameter("s4", [nt, 4, T], bf, isOutput=False)
    w = nc.declare_dram_parameter("w", [128, 23 * 128], bf, isOutput=False)
    y = nc.declare_dram_parameter("y", [nt, 128, 10 * T], bf, isOutput=True)

    segs = _segments(nt)
    nseg = len(segs)

    with TileContext(nc) as tc:
        with (
            tc.tile_pool(name="wpool", bufs=1) as wpool,
            tc.tile_pool(name="xpool", bufs=3) as xpool,
            tc.tile_pool(name="mbpool", bufs=3) as mbpool,
            tc.tile_pool(name="pvpool", bufs=2) as pvpool,
            tc.tile_pool(name="pspool", bufs=2) as pspool,
            tc.tile_pool(name="cpool", bufs=2) as cpool,
            tc.tile_pool(name="ypool", bufs=2) as ypool,
            tc.tile_pool(name="psum", bufs=8, space="PSUM") as psum,
        ):
            wt = wpool.tile([128, 23 * 128], bf)

            def W(i):
                return wt[:, i * 128 : (i + 1) * 128]

            def load(seg):
                t, off, Wd = seg
                xv = x[t].rearrange("p (k t) -> p k t", k=10)[:, :, off : off + Wd]
                mbt = mbpool.tile([128, 4 * T], bf, tag="mb", name="mb_t")[
                    :, : 4 * Wd
                ]
                nc.sync.dma_start(
                    out=mbt.rearrange("p (c t) -> p c t", c=4),
                    in_=s4[t, :, off : off + Wd]
                    .unsqueeze(0)
                    .broadcast_to([128, 4, Wd]),
                )
                xt = xpool.tile([128, 10 * T], bf, tag="xt", name="x_t")[
                    :, : 10 * Wd
                ]
                xtv = xt.rearrange("p (k t) -> p k t", k=10)
                # upper 6 chunks first: pvs (the big DVE op) needs only these
                nc.sync.dma_start(out=xtv[:, 4:, :], in_=xv[:, 4:, :])
                nc.sync.dma_start(out=xtv[:, :4, :], in_=xv[:, :4, :])
                return {"xt": xt, "mbt": mbt, "W": Wd, "seg": seg}

            def mm_into(p, contribs, first, last):
                n = len(contribs)
                for i, (wi, rhs) in enumerate(contribs):
                    nc.tensor.matmul(
                        p,
                        W(wi),
                        rhs,
                        start=(first and i == 0),
                        stop=(last and i == n - 1),
                    )

            def stage_g(st):
                # g = x0?' @ Wg (unscaled), both parities into one [2W]
                # psum, one Act copy.  Ready a full seg before t3 uses it.
                xt, Wd = st["xt"], st["W"]
                sgp = cpool.tile([128, 2 * T], bf, tag="sg", name="sg_t", bufs=4)[
                    :, : 2 * Wd
                ]
                gp = psum.tile([128, 2 * T], f32, tag="psg", name="psg_t", bufs=1)[
                    :, : 2 * Wd
                ]
                for i, (wb, xg0) in enumerate(((12, 2), (17, 0))):
                    mm_into(
                        gp[:, i * Wd : (i + 1) * Wd],
                        [
                            (wb + 0, xt[:, xg0 * Wd : (xg0 + 1) * Wd]),
                            (wb + 1, xt[:, (xg0 + 1) * Wd : (xg0 + 2) * Wd]),
                        ],
                        True,
                        True,
                    )
                nc.scalar.copy(out=sgp, in_=gp)
                st["sgp"] = sgp

            def stage_a(st):
                t, off, Wd = st["seg"]
                xt, mbt = st["xt"], st["mbt"]

                # pvs[m,k]: x1[k] * mb[m] for m in {s,v0,v1,v2}, k in
                # {x1e_0..2, x1o_0..2} -- ONE 24W DVE op.
                pvs = pvpool.tile([128, 24 * T], bf, tag="pv", name="pv_t")[
                    :, : 24 * Wd
                ]
                nc.vector.tensor_mul(
                    pvs.rearrange("p (m k t) -> p m k t", m=4, k=6),
                    xt[:, 4 * Wd :]
                    .rearrange("p (k t) -> p k t", k=6)
                    .unsqueeze(1)
                    .broadcast_to([128, 4, 6, Wd]),
                    mbt.rearrange("p (c t) -> p c t", c=4)
                    .unsqueeze(2)
                    .broadcast_to([128, 4, 6, Wd]),
                )

                def P(m, k):  # block offset helper
                    o = (m * 6 + k) * Wd
                    return pvs[:, o : o + Wd]

                # ps0 = x0 * s (4 chunks)
                ps0 = pspool.tile([128, 4 * T], bf, tag="ps", name="ps_t")[
                    :, : 4 * Wd
                ]
                nc.vector.tensor_mul(
                    ps0.rearrange("p (c t) -> p c t", c=4),
                    xt[:, : 4 * Wd].rearrange("p (c t) -> p c t", c=4),
                    mbt[:, :Wd].unsqueeze(1).broadcast_to([128, 4, Wd]),
                )

                # dta[a] = diag0 + diag1 per parity (a=0: 0o dot over x1e,
                # a=1: 0e dot over x1o); the +diag2 rides the matmul.
                def dpair(m, k0):
                    # blocks {P(m,k0), P(m,k0+3)} -> [128, 2, Wd]
                    o = (m * 6 + k0) * Wd
                    return pvs[:, o : o + 6 * Wd].rearrange(
                        "p (a k t) -> p a k t", a=2, k=3
                    )[:, :, 0, :]

                dta = cpool.tile([128, 2 * T], bf, tag="dta", name="dta_t", bufs=2)[
                    :, : 2 * Wd
                ]
                nc.vector.tensor_add(
                    dta.rearrange("p (a t) -> p a t", a=2), dpair(1, 0), dpair(2, 1)
                )

                # 0e / 0o: 2 m-chunks x 4 K-chunks in a shared [2W] psum
                yt = ypool.tile([128, 10 * T], bf, tag="yo", name="y_t")[
                    :, : 10 * Wd
                ]
                yv = y[t].rearrange("p (k t) -> p k t", k=10)[:, :, off : off + Wd]
                for base, wb, x0c, dgm, da in ((0, 0, 0, 3, 1), (2, 6, 2, 3, 0)):
                    # 0e: x0e chunks 0,1; diag2 = P(3, 5); dta[:, Wd:2Wd]
                    # 0o: x0o chunks 2,3; diag2 = P(3, 2); dta[:, 0:Wd]
                    dg2 = P(3, 5) if base == 0 else P(3, 2)
                    pp = psum.tile(
                        [128, 2 * T], f32, tag="ps0", name="ps0_t", bufs=1
                    )[:, : 2 * Wd]
                    for m in range(2):
                        mm_into(
                            pp[:, m * Wd : (m + 1) * Wd],
                            [
                                (wb + 0 * 2 + m, ps0[:, x0c * Wd : (x0c + 1) * Wd]),
                                (
                                    wb + 1 * 2 + m,
                                    ps0[:, (x0c + 1) * Wd : (x0c + 2) * Wd],
                                ),
                                (wb + 2 * 2 + m, dg2),
                                (wb + 2 * 2 + m, dta[:, da * Wd : (da + 1) * Wd]),
                            ],
                            True,
                            True,
                        )
                    nc.scalar.copy(
                        out=yt[:, base * Wd : (base + 2) * Wd], in_=pp
                    )
                    nc.sync.dma_start(
                        out=yv[:, base : base + 2, :],
                        in_=yt[:, base * Wd : (base + 2) * Wd].rearrange(
                            "p (k t) -> p k t", k=2
                        ),
                    )
                st.update({"pvs": pvs, "yt": yt, "yv": yv})

            def stage_b_dve(st):
                # t3[i,c] = v_c * g_i for both parities in one DVE op
                mbt, sgp, Wd = st["mbt"], st["sgp"], st["W"]
                t3p = cpool.tile([128, 6 * T], bf, tag="t3", name="t3_t", bufs=4)[
                    :, : 6 * Wd
                ]
                nc.vector.tensor_mul(
                    t3p.rearrange("p (i c t) -> p i c t", i=2, c=3),
                    mbt[:, Wd:]
                    .rearrange("p (c t) -> p c t", c=3)
                    .unsqueeze(1)
                    .broadcast_to([128, 2, 3, Wd]),
                    sgp.rearrange("p (i t) -> p i t", i=2)
                    .unsqueeze(2)
                    .broadcast_to([128, 2, 3, Wd]),
                )
                st["t3p"] = t3p

            def stage_b_pe(st):
                pvs, yt, t3p, yv, Wd = (
                    st["pvs"],
                    st["yt"],
                    st["t3p"],
                    st["yv"],
                    st["W"],
                )

                def P(m, k):
                    o = (m * 6 + k) * Wd
                    return pvs[:, o : o + Wd]

                # per parity: k+ x3, k- x3 (N=W), id, h (N=3W) into one
                # [3W] psum; ONE Act copy + ONE store.
                # out1e: x1o products (k base 3), h over x1e*s (pvs 0:3W)
                # out1o: x1e products (k base 0), h over x1o*s (pvs 3W:6W)
                for i, (wb, kb, hoff, ob) in enumerate(
                    ((12, 3, 0, 4), (17, 0, 3, 7))
                ):
                    b3 = psum.tile([128, 3 * T], f32, tag="ps1", name="ps1_t", bufs=1)[
                        :, : 3 * Wd
                    ]
                    # k+ : x1_a * v_b ; k- : x1_b * v_a  (a=c+1, b=c+2 mod 3)
                    # NOTE: accumulation must be slice-major -- interleaving
                    # start/stop groups across slices of one psum region
                    # yields wrong results on hw.
                    for c in range(3):
                        a, b = (c + 1) % 3, (c + 2) % 3
                        mm_into(
                            b3[:, c * Wd : (c + 1) * Wd],
                            [
                                (wb + 3, P(b + 1, kb + a)),
                                (wb + 4, P(a + 1, kb + b)),
                                (22, t3p[:, (i * 3 + c) * Wd : (i * 3 + c + 1) * Wd]),
                                (wb + 2, pvs[:, (hoff + c) * Wd : (hoff + c + 1) * Wd]),
                            ],
                            True,
                            True,
                        )
                    nc.scalar.copy(out=yt[:, ob * Wd : (ob + 3) * Wd], in_=b3)
                    nc.sync.dma_start(
                        out=yv[:, ob : ob + 3, :],
                        in_=yt[:, ob * Wd : (ob + 3) * Wd].rearrange(
                            "p (k t) -> p k t", k=3
                        ),
                    )

            # software pipeline: loads prefetched one seg ahead, stage B
            # (t3 + 1e/1o matmuls + store) one seg behind stage A
            states = {0: load(segs[0])}
            # weights load queued after seg 0's data so the DVE-critical
            # descriptors go out first (PE touches weights later anyway)
            nc.sync.dma_start(out=wt[:, :], in_=w[:, :])
            for i in range(nseg):
                if i + 1 < nseg:
                    states[i + 1] = load(segs[i + 1])
                stage_g(states[i])
                if i >= 1:
                    stage_b_dve(states[i - 1])
                    stage_b_pe(states[i - 1])
                stage_a(states[i])
                if i >= 1:
                    del states[i - 1]
            stage_b_dve(states[nseg - 1])
            stage_b_pe(states[nseg - 1])
    nc.finalize()
    return nc


_PROG_CACHE = {}


def _get_program(Bs):
    if Bs not in _PROG_CACHE:
        _PROG_CACHE[Bs] = _build_program(Bs)
    return _PROG_CACHE[Bs]


def run(inputs, trace=False, **kw):
    in1 = np.asarray(inputs["in1"], np.float32)
    in2 = np.asarray(inputs["in2"], np.float32)
    B = in1.shape[0]
    assert B % (N_CORES * T) == 0, B
    Bs = B // N_CORES

    wpk = _pack_weights(
        np.asarray(inputs["W0e"], np.float32),
        np.asarray(inputs["W0o"], np.float32),
        np.asarray(inputs["W1e"], np.float32),
        np.asarray(inputs["W1o"], np.float32),
    )

    in_maps = []
    for i in range(N_CORES):
        ssl = slice(i * Bs, (i + 1) * Bs)
        xs, s4s = _prep_shard(in1[ssl], in2[ssl])
        in_maps.append({"x": xs, "s4": s4s, "w": wpk})

    nc = _get_program(Bs)
    res = run_bass_kernel_spmd(nc, in_maps, list(range(N_CORES)), trace=trace, **kw)

    out = np.empty((B, 1280), np.float32)
    for i in range(N_CORES):
        out[i * Bs : (i + 1) * Bs] = _post_shard(res.results[i]["y"])
    return out, res


def kernel(**inputs):
    out, _ = run(inputs, trace=False)
    return out


# revision 12
# speedup vs baseline: 1.0399x; 1.0399x over previous
"""Trainium2 Bass kernel for the L1 tensor-product problem.

Math (per batch row b):
  out0e = [x0e*s, CG*(x1o.v)] @ W0e * NORM0E
  out0o = [x0o*s, CG*(x1e.v)] @ W0o * NORM0O
  out1e_c = [CG*x0o*v_c, CG*x1e_c*s, CGC*cross(x1o,v)_c] @ W1e * NORM1E
  out1o_c = [CG*x0e*v_c, CG*x1o_c*s, CGC*cross(x1e,v)_c] @ W1o * NORM1O

Kernel strategy (pure data parallel over batch, 8 cores), v3:
  * bf16 wire + matmul dtype; PSUM accumulates fp32 (rel-err ~4e-3,
    budget 2e-2).
  * DVE is the bottleneck engine (~100% busy): all per-row products run
    as 2x-mode bf16 tensor_tensor ops at the hw max of ~1.92 elem/ns,
    so the schedule minimizes DVE *elements*:
      - pvs: ONE 24T op computes x1 x {s, v0, v1, v2} (the [m,k,t]
        layout puts x1e*s / x1o*s contiguous so the h-path matmul runs
        as a single N=3T instruction, and diag/off-diag blocks land at
        AP-addressable offsets for the k+/k- matmuls and the dots).
      - dots: one DVE add level (dta = d0+d1); the final +d2 is folded
        into the 0e/0o PSUM accumulation as a 4th matmul K-chunk (PE
        has ~13% headroom, DVE has none).
      - t3 = v_c*g on DVE (6T), accumulated via an identity matmul.
  * Unscaled g = x0?' @ Wg runs at the head of each PE seg so sgp is
    ready a full seg before t3 consumes it.
  * The per-parity 1e/1o PSUM is one [128, 3W] region: k+ x3, k- x3,
    one N=3W identity and one N=3W h matmul accumulate into it, and ONE
    Act copy drains it (Act sem ops at ~610ns each were ~27% of the
    Act queue in v2, so fewer, larger copies).
  * First/last tile are split into 4 W=128 subtiles: the DVE starts
    after a quarter-load and the tail drain chain is a quarter-length
    (v2 paid ~5us ramp + ~17us serial drain).
  * Multiplier rows (s,v) reach all 128 partitions via a stride-0
    broadcast DMA read (GpSimd is unusable: its SBUF port contends
    with 2-port DVE ops and its tensor ops trip the chip's utilization
    throttle; PE ones-broadcasts would eat the PE headroom).
"""

import sys

sys.path.insert(0, "/opt/trn_rl_repo")

import numpy as np

import concourse.bass as bass
import concourse.bacc as bacc
import concourse.mybir as mybir
from concourse.bass_utils import run_bass_kernel_spmd
from concourse.tile import TileContext

N_CORES = 8
T = 512  # batch columns per full tile
WEDGE = 128  # subtile width for first/last tile

# irreps: 256x0e + 256x0o + 128x1e + 128x1o
CG = 1.0 / 3.0**0.5
CGC = 1.0 / 6.0**0.5
NORM0E = (1.0 / 384.0) ** 0.5
NORM0O = (1.0 / 384.0) ** 0.5
NORM1E = (3.0 / 512.0) ** 0.5
NORM1O = (3.0 / 512.0) ** 0.5

_BF16 = None


def _bf16():
    global _BF16
    if _BF16 is None:
        import ml_dtypes

        _BF16 = np.dtype(ml_dtypes.bfloat16)
    return _BF16


def _pack_weights(W0e, W0o, W1e, W1o):
    """Fold constants/signs; 22 lhsT chunks [128,128] side by side.

    Order: 0e (kc0m0,kc0m1,kc1m0,kc1m1,kc2m0,kc2m1), 0o (same 6),
    1e (g0,g1,h,k+,k-), 1o (g0,g1,h,k+,k-), identity.
    """
    W0e = W0e.astype(np.float64) * NORM0E
    W0e[256:] *= CG
    W0o = W0o.astype(np.float64) * NORM0O
    W0o[256:] *= CG
    W1e = W1e.astype(np.float64) * NORM1E
    W1e[:384] *= CG
    W1e[384:] *= CGC
    W1o = W1o.astype(np.float64) * NORM1O
    W1o[:384] *= CG
    W1o[384:] *= CGC
    chunks = []
    for W in (W0e, W0o):  # [384, 256]
        for kc in range(3):
            for mc in range(2):
                chunks.append(W[kc * 128 : (kc + 1) * 128, mc * 128 : (mc + 1) * 128])
    for W in (W1e, W1o):  # [512, 128]
        chunks.append(W[0:128, :])      # g0
        chunks.append(W[128:256, :])    # g1
        chunks.append(W[256:384, :])    # h
        chunks.append(W[384:512, :])    # k+
        chunks.append(-W[384:512, :])   # k-
    chunks.append(np.eye(128, dtype=np.float64))  # 22: identity (combine accum)
    packed = np.concatenate(chunks, axis=1)
    return np.ascontiguousarray(packed.astype(_bf16()))


def _prep_shard(in1_s, in2_s):
    """in1 [Bs,1280] -> x [nt, 128, 10*T] bf16; in2 [Bs,4] -> s4 [nt,4,T].

    Chunk order: 0,1=x0e  2,3=x0o  4+c=x1e_c  7+c=x1o_c.
    """
    Bs = in1_s.shape[0]
    nt = Bs // T
    dt = _bf16()
    x = np.empty((nt, 128, 10, T), dt)
    x[:, :, 0:4] = in1_s[:, 0:512].reshape(nt, T, 4, 128).transpose(0, 3, 2, 1)
    x[:, :, 4:7] = in1_s[:, 512:896].reshape(nt, T, 128, 3).transpose(0, 2, 3, 1)
    x[:, :, 7:10] = in1_s[:, 896:1280].reshape(nt, T, 128, 3).transpose(0, 2, 3, 1)
    # multiplier rows in [v0, v1, v2, s] order (pvs m-dim reads them 0..3)
    s4 = np.ascontiguousarray(
        in2_s[:, [1, 2, 3, 0]].reshape(nt, T, 4).transpose(0, 2, 1).astype(dt)
    )
    return np.ascontiguousarray(x.reshape(nt, 128, 10 * T)), s4


def _post_shard(y):
    """Device y [nt, 128, 10*T] bf16 -> [Bs, 1280] fp32 original layout."""
    nt = y.shape[0]
    y = np.asarray(y).reshape(nt, 128, 10, T).astype(np.float32)
    out = np.empty((nt, T, 1280), np.float32)
    out[:, :, 0:512] = y[:, :, 0:4].transpose(0, 3, 2, 1).reshape(nt, T, 512)
    out[:, :, 512:896] = y[:, :, 4:7].transpose(0, 3, 1, 2).reshape(nt, T, 384)
    out[:, :, 896:1280] = y[:, :, 7:10].transpose(0, 3, 1, 2).reshape(nt, T, 384)
    return out.reshape(nt * T, 1280)


def _segments(nt):
    """Subtile the first and last tile so the pipeline ramp and drain
    chains are WEDGE-length instead of T-length."""
    segs = []
    nw = T // WEDGE
    for j in range(nw):
        segs.append((0, j * WEDGE, WEDGE))
    for t in range(1, nt - 1):
        segs.append((t, 0, T))
    if nt > 1:
        for j in range(nw):
            segs.append((nt - 1, j * WEDGE, WEDGE))
    return segs


def _build_program(Bs):
    assert Bs % T == 0, (Bs, T)
    nt = Bs // T
    bf = mybir.dt.bfloat16
    f32 = mybir.dt.float32

    nc = bacc.Bacc()
    x = nc.declare_dram_parameter("x", [nt, 128, 10 * T], bf, isOutput=False)
    s4 = nc.declare_dram_parameter("s4", [nt, 4, T], bf, isOutput=False)
    w = nc.declare_dram_parameter("w", [128, 23 * 128], bf, isOutput=False)
    y = nc.declare_dram_parameter("y", [nt, 128, 10 * T], bf, isOutput=True)

    segs = _segments(nt)
    nseg = len(segs)

    with TileContext(nc) as tc:
        with (
            tc.tile_pool(name="wpool", bufs=1) as wpool,
            tc.tile_pool(name="xpool", bufs=3) as xpool,
            tc.tile_pool(name="mbpool", bufs=3) as mbpool,
            tc.tile_pool(name="pvpool", bufs=2) as pvpool,
            tc.tile_pool(name="pspool", bufs=2) as pspool,
            tc.tile_pool(name="cpool", bufs=2) as cpool,
            tc.tile_pool(name="ypool", bufs=2) as ypool,
            tc.tile_pool(name="psum", bufs=8, space="PSUM") as psum,
        ):
            wt = wpool.tile([128, 23 * 128], bf)

            def W(i):
                return wt[:, i * 128 : (i + 1) * 128]

            def load(seg):
                t, off, Wd = seg
                xv = x[t].rearrange("p (k t) -> p k t", k=10)[:, :, off : off + Wd]
                mbt = mbpool.tile([128, 4 * T], bf, tag="mb", name="mb_t")[
                    :, : 4 * Wd
                ]
                nc.sync.dma_start(
                    out=mbt.rearrange("p (c t) -> p c t", c=4),
                    in_=s4[t, :, off : off + Wd]
                    .unsqueeze(0)
                    .broadcast_to([128, 4, Wd]),
                )
                xt = xpool.tile([128, 10 * T], bf, tag="xt", name="x_t")[
                    :, : 10 * Wd
                ]
                xtv = xt.rearrange("p (k t) -> p k t", k=10)
                # upper 6 chunks first: pvs (the big DVE op) needs only these
                nc.sync.dma_start(out=xtv[:, 4:, :], in_=xv[:, 4:, :])
                nc.sync.dma_start(out=xtv[:, :4, :], in_=xv[:, :4, :])
                return {"xt": xt, "mbt": mbt, "W": Wd, "seg": seg}

            def mm_into(p, contribs, first, last):
                n = len(contribs)
                for i, (wi, rhs) in enumerate(contribs):
                    nc.tensor.matmul(
                        p,
                        W(wi),
                        rhs,
                        start=(first and i == 0),
                        stop=(last and i == n - 1),
                    )

            def stage_g(st):
                # g = x0?' @ Wg (unscaled; only needs xt).  Emitted at the
                # head of each PE seg so sgp is ready a full seg before the
                # t3 op that consumes it.
                xt, Wd = st["xt"], st["W"]
                sgp = cpool.tile([128, 2 * T], bf, tag="sg", name="sg_t", bufs=4)[
                    :, : 2 * Wd
                ]
                for i, (wb, xg0) in enumerate(((12, 2), (17, 0))):
                    gp = psum.tile([128, T], f32, tag="psg", name="psg_t", bufs=2)[
                        :, :Wd
                    ]
                    mm_into(
                        gp,
                        [
                            (wb + 0, xt[:, xg0 * Wd : (xg0 + 1) * Wd]),
                            (wb + 1, xt[:, (xg0 + 1) * Wd : (xg0 + 2) * Wd]),
                        ],
                        True,
                        True,
                    )
                    nc.scalar.copy(out=sgp[:, i * Wd : (i + 1) * Wd], in_=gp)
                st["sgp"] = sgp

            def stage_a(st):
                t, off, Wd = st["seg"]
                xt, mbt = st["xt"], st["mbt"]

                # pvs[m,k]: x1[k] * mb[m] for m in {s,v0,v1,v2}, k in
                # {x1e_0..2, x1o_0..2} -- ONE 24W DVE op.
                pvs = pvpool.tile([128, 24 * T], bf, tag="pv", name="pv_t")[
                    :, : 24 * Wd
                ]
                nc.vector.tensor_mul(
                    pvs.rearrange("p (m k t) -> p m k t", m=4, k=6),
                    xt[:, 4 * Wd :]
                    .rearrange("p (k t) -> p k t", k=6)
                    .unsqueeze(1)
                    .broadcast_to([128, 4, 6, Wd]),
                    mbt.rearrange("p (c t) -> p c t", c=4)
                    .unsqueeze(2)
                    .broadcast_to([128, 4, 6, Wd]),
                )

                def P(m, k):  # block offset helper
                    o = (m * 6 + k) * Wd
                    return pvs[:, o : o + Wd]

                # ps0 = x0 * s (4 chunks)
                ps0 = pspool.tile([128, 4 * T], bf, tag="ps", name="ps_t")[
                    :, : 4 * Wd
                ]
                nc.vector.tensor_mul(
                    ps0.rearrange("p (c t) -> p c t", c=4),
                    xt[:, : 4 * Wd].rearrange("p (c t) -> p c t", c=4),
                    mbt[:, 3 * Wd : 4 * Wd].unsqueeze(1).broadcast_to([128, 4, Wd]),
                )

                # dots: dta = diag0 + diag1, dotp = dta + diag2 per parity
                # (a=0: 0o dot over x1e, a=1: 0e dot over x1o)
                def dpair(m, k0):
                    # blocks {P(m,k0), P(m,k0+3)} -> [128, 2, Wd]
                    o = (m * 6 + k0) * Wd
                    return pvs[:, o : o + 6 * Wd].rearrange(
                        "p (a k t) -> p a k t", a=2, k=3
                    )[:, :, 0, :]

                dta = cpool.tile([128, 2 * T], bf, tag="dta", name="dta_t", bufs=2)[
                    :, : 2 * Wd
                ]
                dotp = cpool.tile([128, 2 * T], bf, tag="dot", name="dot_t", bufs=2)[
                    :, : 2 * Wd
                ]
                dview = lambda ap: ap.rearrange("p (a t) -> p a t", a=2)
                nc.vector.tensor_add(dview(dta), dpair(0, 0), dpair(1, 1))
                nc.vector.tensor_add(dview(dotp), dview(dta), dpair(2, 2))

                # 0e / 0o: 2 m-chunks x 4 K-chunks in a shared [2W] psum
                yt = ypool.tile([128, 10 * T], bf, tag="yo", name="y_t")[
                    :, : 10 * Wd
                ]
                yv = y[t].rearrange("p (k t) -> p k t", k=10)[:, :, off : off + Wd]
                for base, wb, x0c, da in ((0, 0, 0, 1), (2, 6, 2, 0)):
                    # 0e: x0e chunks 0,1 + dot over x1o (dotp a=1)
                    # 0o: x0o chunks 2,3 + dot over x1e (dotp a=0)
                    pp = psum.tile(
                        [128, 2 * T], f32, tag="ps0", name="ps0_t", bufs=2
                    )[:, : 2 * Wd]
                    for m in range(2):
                        mm_into(
                            pp[:, m * Wd : (m + 1) * Wd],
                            [
                                (wb + 0 * 2 + m, ps0[:, x0c * Wd : (x0c + 1) * Wd]),
                                (
                                    wb + 1 * 2 + m,
                                    ps0[:, (x0c + 1) * Wd : (x0c + 2) * Wd],
                                ),
                                (wb + 2 * 2 + m, dotp[:, da * Wd : (da + 1) * Wd]),
                            ],
                            True,
                            True,
                        )
                    nc.scalar.copy(
                        out=yt[:, base * Wd : (base + 2) * Wd], in_=pp
                    )
                    nc.sync.dma_start(
                        out=yv[:, base : base + 2, :],
                        in_=yt[:, base * Wd : (base + 2) * Wd].rearrange(
                            "p (k t) -> p k t", k=2
                        ),
                    )
                st.update({"pvs": pvs, "yt": yt, "yv": yv})

            def stage_b_dve(st):
                # t3[i,c] = v_c * g_i for both parities in one DVE op
                mbt, sgp, Wd = st["mbt"], st["sgp"], st["W"]
                t3p = cpool.tile([128, 6 * T], bf, tag="t3", name="t3_t", bufs=4)[
                    :, : 6 * Wd
                ]
                nc.vector.tensor_mul(
                    t3p.rearrange("p (i c t) -> p i c t", i=2, c=3),
                    mbt[:, : 3 * Wd]
                    .rearrange("p (c t) -> p c t", c=3)
                    .unsqueeze(1)
                    .broadcast_to([128, 2, 3, Wd]),
                    sgp.rearrange("p (i t) -> p i t", i=2)
                    .unsqueeze(2)
                    .broadcast_to([128, 2, 3, Wd]),
                )
                st["t3p"] = t3p

            def stage_b_pe(st):
                pvs, yt, t3p, yv, Wd = (
                    st["pvs"],
                    st["yt"],
                    st["t3p"],
                    st["yv"],
                    st["W"],
                )

                def P(m, k):
                    o = (m * 6 + k) * Wd
                    return pvs[:, o : o + Wd]

                # per parity: k+ x3, k- x3 (N=W), id, h (N=3W) into one
                # [3W] psum; ONE Act copy + ONE store.
                # out1e: x1o products (k base 3), h over x1e*s (pvs 0:3W)
                # out1o: x1e products (k base 0), h over x1o*s (pvs 3W:6W)
                for i, (wb, kb, hoff, ob) in enumerate(
                    ((12, 3, 18, 4), (17, 0, 21, 7))
                ):
                    # k+ : x1_a * v_b ; k- : x1_b * v_a  (a=c+1, b=c+2 mod 3)
                    # NOTE: accumulation must be slice-major -- interleaving
                    # start/stop groups across slices of one psum region
                    # yields wrong results on hw.
                    def contribs(c):
                        a, b = (c + 1) % 3, (c + 2) % 3
                        return [
                            (wb + 3, P(b, kb + a)),
                            (wb + 4, P(a, kb + b)),
                            (22, t3p[:, (i * 3 + c) * Wd : (i * 3 + c + 1) * Wd]),
                            (wb + 2, pvs[:, (hoff + c) * Wd : (hoff + c + 1) * Wd]),
                        ]

                    # components 0,1 share a [2W] psum + one copy; c=2 alone
                    pp = psum.tile([128, 2 * T], f32, tag="ps1", name="ps1_t", bufs=1)[
                        :, : 2 * Wd
                    ]
                    for c in range(2):
                        mm_into(pp[:, c * Wd : (c + 1) * Wd], contribs(c), True, True)
                    pc2 = psum.tile([128, T], f32, tag="psg", name="ps1c_t", bufs=2)[
                        :, :Wd
                    ]
                    mm_into(pc2, contribs(2), True, True)
                    nc.scalar.copy(out=yt[:, ob * Wd : (ob + 2) * Wd], in_=pp)
                    nc.scalar.copy(
                        out=yt[:, (ob + 2) * Wd : (ob + 3) * Wd], in_=pc2
                    )
                    nc.sync.dma_start(
                        out=yv[:, ob : ob + 3, :],
                        in_=yt[:, ob * Wd : (ob + 3) * Wd].rearrange(
                            "p (k t) -> p k t", k=3
                        ),
                    )

            # software pipeline: loads prefetched one seg ahead, stage B
            # (t3 + 1e/1o matmuls + store) one seg behind stage A
            states = {0: load(segs[0])}
            # weights load queued after seg 0's data so the DVE-critical
            # descriptors go out first (PE touches weights later anyway)
            nc.sync.dma_start(out=wt[:, :], in_=w[:, :])
            for i in range(nseg):
                if i + 1 < nseg:
                    states[i + 1] = load(segs[i + 1])
                stage_g(states[i])
                if i >= 1:
                    stage_b_dve(states[i - 1])
                    stage_b_pe(states[i - 1])
                stage_a(states[i])
                if i >= 1:
                    del states[i - 1]
            stage_b_dve(states[nseg - 1])
            stage_b_pe(states[nseg - 1])
    nc.finalize()
    return nc


_PROG_CACHE = {}


def _get_program(Bs):
    if Bs not in _PROG_CACHE:
        _PROG_CACHE[Bs] = _build_program(Bs)
    return _PROG_CACHE[Bs]


def run(inputs, trace=False, **kw):
    in1 = np.asarray(inputs["in1"], np.float32)
    in2 = np.asarray(inputs["in2"], np.float32)
    B = in1.shape[0]
    assert B % (N_CORES * T) == 0, B
    Bs = B // N_CORES

    wpk = _pack_weights(
        np.asarray(inputs["W0e"], np.float32),
        np.asarray(inputs["W0o"], np.float32),
        np.asarray(inputs["W1e"], np.float32),
        np.asarray(inputs["W1o"], np.float32),
    )

    in_maps = []
    for i in range(N_CORES):
        ssl = slice(i * Bs, (i + 1) * Bs)
        xs, s4s = _prep_shard(in1[ssl], in2[ssl])
        in_maps.append({"x": xs, "s4": s4s, "w": wpk})

    nc = _get_program(Bs)
    res = run_bass_kernel_spmd(nc, in_maps, list(range(N_CORES)), trace=trace, **kw)

    out = np.empty((B, 1280), np.float32)
    for i in range(N_CORES):
        out[i * Bs : (i + 1) * Bs] = _post_shard(res.results[i]["y"])
    return out, res


def kernel(**inputs):
    out, _ = run(inputs, trace=False)
    return out
